# revision 3
# baseline (speedup 1.0000x reference)
"""BufferedCIF module on 8 Trainium2 NeuronCores (Bass/Tile).

Strategy (sharding_hint: data-parallel over batch + sequence split with halo):
  Core c handles batch b=c//2, sequence-half h=c%2.
  Phase 1: per-core conv1d(H->C,k=5)+LayerNorm+ReLU+Linear+sigmoid over its
    half (halo k//2) -> alpha half.  Host gathers alpha (32KB/core).
  Phase 2: every core runs the full-sequence CIF fire scan from alpha using an
    exact-arithmetic formulation (Veltkamp-split two-limb cumsums + floor
    refinement -> bit-identical fire decisions to the sequential f32 scan),
    then computes its half of emits as segment-selection matmuls
    (emits_tile = M^T.T @ x with M built on-device from the scan state).

kernel(**inputs) takes the FULL inputs and returns (emits, fires, alphas).
"""
import numpy as np

import concourse.bass as bass
import concourse.tile as tile
from concourse import mybir
from concourse.bass_utils import run_bass_kernel_spmd

F32 = mybir.dt.float32
I32 = mybir.dt.int32
AF = mybir.ActivationFunctionType
OP = mybir.AluOpType
AX = mybir.AxisListType

B, T, H, C, K = 4, 16384, 512, 256, 5
HALF = T // 2
NTILE = HALF // 128
XT_COLS = HALF + K - 1
LN_EPS = 1e-5
BIG = 1048576.0

# --------------------------------------------------------------------------
# Walrus in this environment rejects any instruction carrying more than one
# semaphore wait; split extras onto same-engine NoOps (queues are in-order).
import bass_rust
from concourse import tile as _tile
from concourse.tile import ScopedClock

_MAX_WAITS = 1


def _patched_drain_and_barrier(self, tick_clock, wait_clock):
    nc = self.nc
    gc = tick_clock.global_clock
    ticks = [gc[i] for i in range(27)]
    nz = [i for i, t in enumerate(ticks) if t > 0]
    for c0 in range(0, len(nz), _MAX_WAITS):
        chunk = nz[c0:c0 + _MAX_WAITS]
        part = [0] * 27
        for i in chunk:
            part[i] = ticks[i]
        nop = nc.sync.nop(nofuse=True, hint="split_drain_wait")
        wait_clock.add_sem_waits(nop.ins, ScopedClock({None: bass_rust.VectorClock(part)}))
    nc.sync.drain()

    nc.all_engine_barrier()
    assert self.sems is not None
    popped = nc._tile_sem_poison_stack.pop()
    assert popped is self._sem_poison
    nc.clear_and_free_semaphores(list(self.sems.allocated().values()))
    nc.all_engine_barrier()


_tile.TileContext._drain_and_barrier = _patched_drain_and_barrier

if getattr(_tile.TileContext._commit_and_lower, "_split_patch", False):
    _orig_commit_and_lower = _tile.TileContext._commit_and_lower.__wrapped_orig__
else:
    _orig_commit_and_lower = _tile.TileContext._commit_and_lower


def _commit_and_lower_split(self, inst, original_block, old_bb_map, bb_to_exit_bb):
    si = getattr(inst, "sync_info", None)
    if si is not None and type(inst).__name__ not in ("InstEventSemaphore",):
        waits = list(si.on_wait)
        if len(waits) > 1:
            for w in waits[:-1]:
                nop = mybir.InstNoOp(
                    name=self.nc.get_next_instruction_name(), ins=[], outs=[])
                nop.engine = inst.engine
                nop.sync_info = bass_rust.SyncInfo(on_wait=[w], on_update=[])
                self._add_instruction(nop)
            si.on_wait = [waits[-1]]
            inst.sync_info = si
    return _orig_commit_and_lower(self, inst, original_block, old_bb_map,
                                  bb_to_exit_bb)


_commit_and_lower_split._split_patch = True
_commit_and_lower_split.__wrapped_orig__ = _orig_commit_and_lower
_tile.TileContext._commit_and_lower = _commit_and_lower_split
# --------------------------------------------------------------------------


def _act_raw(nc, out, in_, func, scale=1.0):
    eng = nc.scalar
    inputs = [eng.lower_ap(in_)]
    bias = nc.const_aps.scalar_like(0.0, in_)
    for arg in [bias, scale, 0.0]:
        if isinstance(arg, bass.AP):
            inputs.append(eng.lower_ap(arg))
        else:
            inputs.append(mybir.ImmediateValue(dtype=mybir.dt.float32, value=arg))
    return eng.add_instruction(mybir.InstActivation(
        name=nc.get_next_instruction_name(),
        func=func, ins=inputs, outs=[eng.lower_ap(out)]))


def build_alpha_kernel():
    nc = bass.Bass()
    xt = nc.dram_tensor("xt", [H, XT_COLS], F32, kind="ExternalInput")
    wk = nc.dram_tensor("wk", [20, 128, C], F32, kind="ExternalInput")
    cvec = nc.dram_tensor("cvec", [1, 4 * C], F32, kind="ExternalInput")
    linb = nc.dram_tensor("linb", [1, 1], F32, kind="ExternalInput")
    ah = nc.dram_tensor("ah", [HALF], F32, kind="ExternalOutput")

    with tile.TileContext(nc) as tc:
        with (
            tc.tile_pool(name="per", bufs=1) as per,
            tc.tile_pool(name="wrk", bufs=3) as wrk,
            tc.tile_pool(name="cps", bufs=4, space="PSUM") as cps,
            tc.tile_pool(name="psb", bufs=2, space="PSUM") as psb,
        ):
            xts = []
            for hc in range(4):
                xtile = per.tile([128, XT_COLS], F32, tag=f"xt{hc}")
                nc.sync.dma_start(xtile[:], xt[128 * hc:128 * (hc + 1), :])
                xts.append(xtile)
            wks = []
            for i in range(20):
                wtile = per.tile([128, C], F32, tag=f"wk{i}")
                nc.sync.dma_start(wtile[:], wk[i, :, :])
                wks.append(wtile)

            ones_row = per.tile([1, 128], F32, tag="ones_row")
            nc.vector.memset(ones_row[:], 1.0)
            cvec_sb = per.tile([1, 4 * C], F32, tag="cvec_sb")
            nc.sync.dma_start(cvec_sb[:], cvec[:, :])
            bc_ps = psb.tile([128, 512], F32, tag="scratch")
            bc_ps2 = psb.tile([128, 512], F32, tag="scratch")
            nc.tensor.matmul(bc_ps[:], ones_row[:], cvec_sb[:, 0:512],
                             start=True, stop=True)
            nc.tensor.matmul(bc_ps2[:], ones_row[:], cvec_sb[:, 512:1024],
                             start=True, stop=True)
            bcast = per.tile([128, 4 * C], F32, tag="bcast")
            nc.vector.tensor_copy(bcast[:, 0:512], bc_ps[:])
            nc.vector.tensor_copy(bcast[:, 512:1024], bc_ps2[:])
            cbr = bcast[:, 0:C]
            lnwr = bcast[:, C:2 * C]
            lnbr = bcast[:, 2 * C:3 * C]
            linwr = bcast[:, 3 * C:4 * C]

            linb_sb = per.tile([1, 1], F32, tag="linb_sb")
            nc.sync.dma_start(linb_sb[:], linb[:, :])
            lb_ps = psb.tile([128, 512], F32, tag="scratch")
            nc.tensor.matmul(lb_ps[:, 0:1], ones_row[:], linb_sb[:],
                             start=True, stop=True)
            linb_col = per.tile([128, 1], F32, tag="linb_col")
            nc.vector.tensor_copy(linb_col[:], lb_ps[:, 0:1])

            stage = per.tile([128, NTILE], F32, tag="stage")

            for j in range(NTILE):
                ps_c = cps.tile([128, C], F32, tag="conv")
                first = True
                for k in range(K):
                    for hc in range(4):
                        nc.tensor.matmul(
                            ps_c[:],
                            xts[hc][:, 128 * j + k:128 * j + k + 128],
                            wks[hc * K + k][:],
                            start=first,
                            stop=(k == K - 1 and hc == 3),
                        )
                        first = False
                yb = wrk.tile([128, C], F32, tag="yb")
                nc.vector.tensor_tensor(yb[:], ps_c[:], cbr, op=OP.add)
                mu = wrk.tile([128, 1], F32, tag="mu")
                nc.vector.tensor_reduce(mu[:], yb[:], AX.X, OP.add)
                nc.vector.tensor_scalar_mul(mu[:], mu[:], 1.0 / C)
                yc = wrk.tile([128, C], F32, tag="yc")
                nc.vector.tensor_scalar(yc[:], yb[:], mu[:], None, op0=OP.subtract)
                sq = wrk.tile([128, C], F32, tag="sq")
                nc.vector.tensor_mul(sq[:], yc[:], yc[:])
                var = wrk.tile([128, 1], F32, tag="var")
                nc.vector.tensor_reduce(var[:], sq[:], AX.X, OP.add)
                nc.vector.tensor_scalar_mul(var[:], var[:], 1.0 / C)
                ve = wrk.tile([128, 1], F32, tag="ve")
                nc.vector.tensor_scalar(ve[:], var[:], LN_EPS, None, op0=OP.add)
                r0 = wrk.tile([128, 1], F32, tag="r0")
                _act_raw(nc, r0[:], ve[:], AF.Rsqrt)
                nwt = wrk.tile([128, 1], F32, tag="nwt")
                nc.vector.tensor_mul(nwt[:], ve[:], r0[:])
                nc.vector.tensor_mul(nwt[:], nwt[:], r0[:])
                nc.vector.tensor_scalar(nwt[:], nwt[:], -0.5, 1.5,
                                        op0=OP.mult, op1=OP.add)
                rstd = wrk.tile([128, 1], F32, tag="rstd")
                nc.vector.tensor_mul(rstd[:], r0[:], nwt[:])
                yn = wrk.tile([128, C], F32, tag="yn")
                nc.vector.tensor_scalar(yn[:], yc[:], rstd[:], None, op0=OP.mult)
                nc.vector.tensor_mul(yn[:], yn[:], lnwr)
                nc.vector.tensor_tensor(yn[:], yn[:], lnbr, op=OP.add)
                nc.vector.tensor_scalar_max(yn[:], yn[:], 0.0)
                nc.vector.tensor_mul(yn[:], yn[:], linwr)
                z = wrk.tile([128, 1], F32, tag="z")
                nc.vector.tensor_reduce(z[:], yn[:], AX.X, OP.add)
                nc.vector.tensor_tensor(z[:], z[:], linb_col[:], op=OP.add)
                ez = wrk.tile([128, 1], F32, tag="ez")
                nc.scalar.activation(ez[:], z[:], AF.Exp, scale=-1.0)
                nc.vector.tensor_scalar(ez[:], ez[:], 1.0, None, op0=OP.add)
                nc.vector.reciprocal(stage[:, j:j + 1], ez[:])

            nc.sync.dma_start(ah[:].rearrange("(j p) -> p j", p=128), stage[:])
    return nc


def build_scan_emit_kernel():
    nc = bass.Bass()
    alpha = nc.dram_tensor("alpha", [T], F32, kind="ExternalInput")
    xin = nc.dram_tensor("xin", [128 + HALF, H], F32, kind="ExternalInput")
    selt = nc.dram_tensor("selt", [128, 128], F32, kind="ExternalInput")
    emits = nc.dram_tensor("emits", [HALF, H], F32, kind="ExternalOutput")
    firesf = nc.dram_tensor("firesf", [T], F32, kind="ExternalOutput")
    asum = nc.dram_tensor("asum", [1, 1], F32, kind="ExternalOutput")

    with tile.TileContext(nc) as tc:
        with (
            tc.tile_pool(name="per", bufs=1) as per,
            tc.tile_pool(name="xp", bufs=3) as xp,
            tc.tile_pool(name="lt", bufs=3) as lt,
            tc.tile_pool(name="bps", bufs=2, space="PSUM") as bps,
            tc.tile_pool(name="eps", bufs=2, space="PSUM") as eps_pool,
            tc.tile_pool(name="sps", bufs=2, space="PSUM") as sps,
        ):
            a_sb = per.tile([128, 128], F32, tag="a_sb")
            nc.sync.dma_start(a_sb[:], alpha[:].rearrange("(p f) -> p f", p=128))
            selt_sb = per.tile([128, 128], F32, tag="selt_sb")
            nc.sync.dma_start(selt_sb[:], selt[:, :])

            # Veltkamp split: a = hi + lo (hi on a coarse grid so all chunk
            # cumsums below are exact in f32)
            c1 = per.tile([128, 128], F32, tag="c1")
            nc.vector.tensor_scalar_mul(c1[:], a_sb[:], 4097.0)
            t1 = per.tile([128, 128], F32, tag="t1")
            nc.vector.tensor_sub(t1[:], c1[:], a_sb[:])
            hi = per.tile([128, 128], F32, tag="hi")
            nc.vector.tensor_sub(hi[:], c1[:], t1[:])
            lo = per.tile([128, 128], F32, tag="lo")
            nc.vector.tensor_sub(lo[:], a_sb[:], hi[:])

            R_hi = per.tile([128, 129], F32, tag="R_hi")
            nc.vector.memset(R_hi[:, 0:1], 0.0)
            nc.vector.tensor_tensor_scan(R_hi[:, 1:129], hi[:], hi[:], 0.0,
                                         op0=OP.add, op1=OP.bypass)
            R_lo = per.tile([128, 129], F32, tag="R_lo")
            nc.vector.memset(R_lo[:, 0:1], 0.0)
            nc.vector.tensor_tensor_scan(R_lo[:, 1:129], lo[:], lo[:], 0.0,
                                         op0=OP.add, op1=OP.bypass)

            thi_i = per.tile([128, 1], I32, tag="thi_i")
            nc.vector.tensor_copy(thi_i[:], R_hi[:, 128:129])
            rhs3 = per.tile([128, 3], F32, tag="rhs3")
            nc.vector.tensor_copy(rhs3[:, 0:1], thi_i[:])
            nc.vector.tensor_sub(rhs3[:, 1:2], R_hi[:, 128:129], rhs3[:, 0:1])
            nc.vector.tensor_copy(rhs3[:, 2:3], R_lo[:, 128:129])

            ii = per.tile([128, 128], I32, tag="ii")
            nc.gpsimd.iota(ii[:], pattern=[[-1, 128]], base=0, channel_multiplier=1)
            TRI = per.tile([128, 128], F32, tag="TRI")
            nc.vector.tensor_scalar(TRI[:], ii[:], 0, None, op0=OP.is_lt)
            ID = per.tile([128, 128], F32, tag="ID")
            nc.vector.tensor_scalar(ID[:], ii[:], 0, None, op0=OP.is_equal)

            p3_ps = sps.tile([128, 128], F32, tag="scratch")
            nc.tensor.matmul(p3_ps[:, 0:3], TRI[:], rhs3[:], start=True, stop=True)
            P3 = per.tile([128, 3], F32, tag="P3")
            nc.vector.tensor_copy(P3[:], p3_ps[:, 0:3])

            pf_i = per.tile([128, 1], I32, tag="pf_i")
            nc.vector.tensor_copy(pf_i[:], P3[:, 1:2])
            pf_if = per.tile([128, 1], F32, tag="pf_if")
            nc.vector.tensor_copy(pf_if[:], pf_i[:])
            d0 = per.tile([128, 1], F32, tag="d0")
            nc.vector.tensor_sub(d0[:], P3[:, 1:2], pf_if[:])
            F_base = per.tile([128, 1], F32, tag="F_base")
            nc.vector.tensor_add(F_base[:], P3[:, 0:1], pf_if[:])

            L = per.tile([128, 129], F32, tag="L")
            nc.vector.tensor_scalar(L[:], R_hi[:], d0[:], None, op0=OP.add)
            losum = per.tile([128, 129], F32, tag="losum")
            nc.vector.tensor_scalar(losum[:], R_lo[:], P3[:, 2:3], None, op0=OP.add)
            Tt = per.tile([128, 129], F32, tag="Tt")
            nc.vector.tensor_add(Tt[:], L[:], losum[:])
            f1_i = per.tile([128, 129], I32, tag="f1_i")
            nc.vector.tensor_copy(f1_i[:], Tt[:])
            F1f = per.tile([128, 129], F32, tag="F1f")
            nc.vector.tensor_copy(F1f[:], f1_i[:])
            d = per.tile([128, 129], F32, tag="d")
            nc.vector.tensor_sub(d[:], L[:], F1f[:])
            nc.vector.tensor_add(d[:], d[:], losum[:])
            lt0 = per.tile([128, 129], F32, tag="lt0")
            nc.vector.tensor_scalar(lt0[:], d[:], 0.0, None, op0=OP.is_lt)
            ge1 = per.tile([128, 129], F32, tag="ge1")
            nc.vector.tensor_scalar(ge1[:], d[:], 1.0, None, op0=OP.is_ge)
            F_rel = per.tile([128, 129], F32, tag="F_rel")
            nc.vector.tensor_add(F_rel[:], F1f[:], ge1[:])
            nc.vector.tensor_sub(F_rel[:], F_rel[:], lt0[:])
            d_fix = per.tile([128, 129], F32, tag="d_fix")
            nc.vector.tensor_sub(d_fix[:], d[:], ge1[:])
            nc.vector.tensor_add(d_fix[:], d_fix[:], lt0[:])
            F_ext = per.tile([128, 129], F32, tag="F_ext")
            nc.vector.tensor_scalar(F_ext[:], F_rel[:], F_base[:], None, op0=OP.add)

            fire = per.tile([128, 128], F32, tag="fire")
            nc.vector.tensor_sub(fire[:], F_ext[:, 1:129], F_ext[:, 0:128])
            a_u1 = per.tile([128, 128], F32, tag="a_u1")
            nc.vector.tensor_scalar(a_u1[:], d_fix[:, 0:128], -1.0, 1.0,
                                    op0=OP.mult, op1=OP.add)
            u = per.tile([128, 128], F32, tag="u")
            nc.vector.tensor_mul(u[:], fire[:], a_u1[:])
            nc.vector.tensor_sub(u[:], a_sb[:], u[:])
            au1f = per.tile([128, 128], F32, tag="au1f")
            nc.vector.tensor_mul(au1f[:], a_u1[:], fire[:])

            G = per.tile([128, 128], F32, tag="G")
            nc.vector.tensor_scalar(G[:], F_ext[:, 1:129], BIG - 1.0, None, op0=OP.add)
            fb = per.tile([128, 128], F32, tag="fb")
            nc.vector.tensor_scalar_mul(fb[:], fire[:], BIG)
            nc.vector.tensor_sub(G[:], G[:], fb[:])

            ones_col = per.tile([128, 1], F32, tag="ones_col")
            nc.vector.memset(ones_col[:], 1.0)
            ared = per.tile([128, 1], F32, tag="ared")
            nc.vector.tensor_reduce(ared[:], a_sb[:], AX.X, OP.add)
            as_ps = sps.tile([128, 128], F32, tag="scratch")
            nc.tensor.matmul(as_ps[0:1, 0:1], ared[:], ones_col[:],
                             start=True, stop=True)
            as_sb = per.tile([1, 1], F32, tag="as_sb")
            nc.vector.tensor_copy(as_sb[:], as_ps[0:1, 0:1])
            nc.sync.dma_start(asum[:, :], as_sb[:])

            nc.sync.dma_start(firesf[:].rearrange("(p f) -> p f", p=128), fire[:])

            # permute the 65 relevant chunk rows (prev + 64 own) to canonical
            # positions: perm = SELT.T @ X (SELT column i selects chunk
            # own_start-1+i; out-of-range -> zero row)
            def permute(src, tag):
                pps = sps.tile([128, 128], F32, tag="scratch")
                nc.tensor.matmul(pps[:], selt_sb[:], src[:], start=True, stop=True)
                out = per.tile([128, 128], F32, tag=tag)
                nc.vector.tensor_copy(out[:], pps[:])
                return out

            Gp = permute(G, "Gp")
            Gflat = per.tile([1, 65 * 128], F32, tag="Gflat")
            nc.sync.dma_start(Gflat[:], Gp[0:65, :])
            Fp = permute(F_ext[:, 1:129], "Fp")
            up = permute(u, "up")
            au1p = permute(au1f, "au1p")

            def transposeT(src, tag):
                tps = sps.tile([128, 128], F32, tag="scratch")
                nc.tensor.transpose(tps[:], src[:], ID[:])
                out = per.tile([128, 128], F32, tag=tag)
                nc.vector.tensor_copy(out[:], tps[:])
                return out

            FT = transposeT(Fp, "FT")
            uT = transposeT(up, "uT")
            au1T = transposeT(au1p, "au1T")

            ones_row = per.tile([1, 128], F32, tag="ones_row")
            nc.vector.memset(ones_row[:], 1.0)

            x_prev = xp.tile([128, H], F32, tag="x")
            nc.sync.dma_start(x_prev[:], xin[0:128, :])

            for j in range(NTILE):
                brow = bps.tile([128, 128], F32, tag="brow")
                nc.tensor.matmul(brow[:], ones_row[:],
                                 Gflat[:, 128 * (j + 1):128 * (j + 2)],
                                 start=True, stop=True)
                lhc = lt.tile([128, 128], F32, tag="lhc")
                nc.vector.tensor_scalar(lhc[:], brow[:], FT[:, j + 1:j + 2], None,
                                        op0=OP.is_equal)
                nc.vector.tensor_scalar(lhc[:], lhc[:], uT[:, j + 1:j + 2], None,
                                        op0=OP.mult)
                dg = lt.tile([128, 128], F32, tag="dg")
                nc.vector.tensor_scalar(dg[:], ID[:], au1T[:, j + 1:j + 2], None,
                                        op0=OP.mult)
                nc.vector.tensor_add(lhc[:], lhc[:], dg[:])
                lhp = lt.tile([128, 128], F32, tag="lhp")
                nc.vector.tensor_scalar(lhp[:], brow[:], FT[:, j:j + 1], None,
                                        op0=OP.is_equal)
                nc.vector.tensor_scalar(lhp[:], lhp[:], uT[:, j:j + 1], None,
                                        op0=OP.mult)

                x_cur = xp.tile([128, H], F32, tag="x")
                nc.sync.dma_start(x_cur[:], xin[128 * (j + 1):128 * (j + 2), :])

                ps_e = eps_pool.tile([128, H], F32, tag="pse")
                nc.tensor.matmul(ps_e[:], lhp[:], x_prev[:], start=True, stop=False)
                nc.tensor.matmul(ps_e[:], lhc[:], x_cur[:], start=False, stop=True)

                es = lt.tile([128, H], F32, tag="es")
                nc.vector.tensor_copy(es[:], ps_e[:])
                nc.sync.dma_start(emits[128 * j:128 * (j + 1), :], es[:])
                x_prev = x_cur
    return nc


def _prep_phase1(inputs):
    x = np.asarray(inputs["encoder_out"], np.float32)
    conv_w = np.asarray(inputs["conv_w"], np.float32)
    wk = np.empty((20, 128, C), np.float32)
    for hc in range(4):
        for k in range(K):
            wk[hc * K + k] = conv_w[:, 128 * hc:128 * (hc + 1), k].T
    cvec = np.concatenate([
        np.asarray(inputs["conv_b"], np.float32),
        np.asarray(inputs["ln_w"], np.float32),
        np.asarray(inputs["ln_b"], np.float32),
        np.asarray(inputs["lin_w"], np.float32),
    ]).reshape(1, 4 * C)
    linb = np.asarray(inputs["lin_b"], np.float32).reshape(1, 1)
    in_maps = []
    for c in range(8):
        b, hh = c // 2, c % 2
        lo_f = hh * HALF - (K // 2)
        hi_f = hh * HALF + HALF + (K // 2)
        seg = np.zeros((XT_COLS, H), np.float32)
        s0, s1 = max(lo_f, 0), min(hi_f, T)
        seg[s0 - lo_f:s1 - lo_f] = x[b, s0:s1]
        xt = np.ascontiguousarray(seg.T)
        in_maps.append({"xt": xt, "wk": wk, "cvec": cvec, "linb": linb})
    return in_maps


def _prep_phase2(inputs, alpha_full):
    x = np.asarray(inputs["encoder_out"], np.float32)
    in_maps = []
    for c in range(8):
        b, hh = c // 2, c % 2
        lo_f = hh * HALF - 128
        seg = np.zeros((128 + HALF, H), np.float32)
        s0 = max(lo_f, 0)
        seg[s0 - lo_f:] = x[b, s0:hh * HALF + HALF]
        selt = np.zeros((128, 128), np.float32)
        for i in range(65):
            p = 64 * hh - 1 + i
            if 0 <= p < 128:
                selt[p, i] = 1.0
        in_maps.append({"alpha": np.ascontiguousarray(alpha_full[b]),
                        "xin": seg, "selt": selt})
    return in_maps


_cache = {}


def kernel(**inputs):
    if "nc1" not in _cache:
        _cache["nc1"] = build_alpha_kernel()
        _cache["nc2"] = build_scan_emit_kernel()
    nc1, nc2 = _cache["nc1"], _cache["nc2"]

    res1 = run_bass_kernel_spmd(nc1, _prep_phase1(inputs), core_ids=list(range(8)))
    alpha_full = np.empty((B, T), np.float32)
    for c in range(8):
        b, hh = c // 2, c % 2
        alpha_full[b, hh * HALF:(hh + 1) * HALF] = res1.results[c]["ah"]

    res2 = run_bass_kernel_spmd(nc2, _prep_phase2(inputs, alpha_full),
                                core_ids=list(range(8)))
    emits = np.empty((B, T, H), np.float32)
    fires = np.empty((B, T), bool)
    alphas = np.empty((B, 1, 1), np.float32)
    for c in range(8):
        b, hh = c // 2, c % 2
        r = res2.results[c]
        emits[b, hh * HALF:(hh + 1) * HALF] = r["emits"]
        if hh == 0:
            fires[b] = r["firesf"] != 0.0
            alphas[b, 0, 0] = r["asum"][0, 0]
    return emits, fires, alphas
